# revision 2
# baseline (speedup 1.0000x reference)
"""Multi-head causal self-attention (b=4, n=2048, d=1024, 16 heads) on 8 TRN2 cores.

Sharding: core i handles batch b = i//2 and head-group g = i%2 (8 heads each).
Per core: QKV projections for its head-group, flash-style causal attention in
scoresT [k, q] layout (bf16 matmuls, fp32 PSUM accum), and a partial transposed
output projection outT = Wo_rows.T-contract over the local 512 ctx dims.
Host: out[b] = (outT[2b] + outT[2b+1]).T + bo.
"""
import sys

for _p in ("/opt/trn_rl_repo", "/root/.axon_site", "/root/.axon_site/_ro/trn_rl_repo",
           "/root/.axon_site/_ro/pypackages"):
    if _p not in sys.path:
        sys.path.append(_p)

import numpy as np
import ml_dtypes

import concourse.bacc as bacc
import concourse.tile as tile
from concourse import mybir
from concourse.bass_utils import run_bass_kernel_spmd

BF16 = mybir.dt.bfloat16
F32 = mybir.dt.float32

N = 2048          # sequence length
D_IN = 1024       # model dim
D_LOC = 512       # per-core head-group width (8 heads * 64)
HD = 64           # head dim
NPAIRS = 4        # head pairs per core
NT = 4            # q tiles of 512
NKC = 16          # k chunks of 128
SCALE = 1.0 / 8.0  # 1/sqrt(HD)

EXP = mybir.ActivationFunctionType.Exp
MULT = mybir.AluOpType.mult


def _build_program():
    nc = bacc.Bacc("TRN2", target_bir_lowering=False, debug=False, num_devices=8)

    xT = nc.dram_tensor("xT", [D_IN, N], BF16, kind="ExternalInput").ap()
    wq = nc.dram_tensor("wq", [D_IN, D_LOC], BF16, kind="ExternalInput").ap()
    wk = nc.dram_tensor("wk", [D_IN, D_LOC], BF16, kind="ExternalInput").ap()
    wv = nc.dram_tensor("wv", [D_IN, D_LOC], BF16, kind="ExternalInput").ap()
    wo = nc.dram_tensor("wo", [D_LOC, D_IN], BF16, kind="ExternalInput").ap()
    masks = nc.dram_tensor("masks", [128, 2048], BF16, kind="ExternalInput").ap()
    outT = nc.dram_tensor("outT", [D_IN, N], F32, kind="ExternalOutput").ap()

    with tile.TileContext(nc) as tc:
        with tc.tile_pool(name="persist", bufs=1) as pp, \
             tc.tile_pool(name="qkv", bufs=2) as qkvp, \
             tc.tile_pool(name="exp", bufs=3) as ep, \
             tc.tile_pool(name="small", bufs=2) as sp, \
             tc.tile_pool(name="evac", bufs=3) as evp, \
             tc.tile_pool(name="ps_s", bufs=2, space="PSUM") as ps_s_pool, \
             tc.tile_pool(name="ps_c", bufs=1, space="PSUM") as ps_c_pool, \
             tc.tile_pool(name="ps_m", bufs=1, space="PSUM") as ps_m_pool, \
             tc.tile_pool(name="ps_p", bufs=2, space="PSUM") as ps_p_pool:

            # ---- persistent SBUF loads ----
            xT_sb = []
            for i in range(8):
                t = pp.tile([128, N], BF16, tag=f"xT{i}", name=f"xT{i}")
                nc.sync.dma_start(t[:], xT[i * 128:(i + 1) * 128, :])
                xT_sb.append(t)
            w_sb = {}
            for name, src in (("wq", wq), ("wk", wk), ("wv", wv)):
                tiles = []
                for i in range(8):
                    t = pp.tile([128, D_LOC], BF16, tag=f"{name}{i}", name=f"{name}{i}")
                    nc.sync.dma_start(t[:], src[i * 128:(i + 1) * 128, :])
                    tiles.append(t)
                w_sb[name] = tiles
            wo_sb = []
            for i in range(4):
                t = pp.tile([128, D_IN], BF16, tag=f"wo{i}", name=f"wo{i}")
                nc.sync.dma_start(t[:], wo[i * 128:(i + 1) * 128, :])
                wo_sb.append(t)
            mask_sb = pp.tile([128, 2048], BF16, tag="masks")
            nc.sync.dma_start(mask_sb[:], masks[:])

            ones_bf = pp.tile([128, 1], BF16, tag="ones_bf")
            nc.vector.memset(ones_bf[:], 1.0)
            ones_f = pp.tile([33, 64], F32, tag="ones_f")
            nc.vector.memset(ones_f[:], 1.0)

            ctxT_sb = [pp.tile([128, N], BF16, tag=f"ctxT{p}", name=f"ctxT{p}") for p in range(NPAIRS)]

            # ---- per head-pair: projections then attention ----
            for p in range(NPAIRS):
                pc = slice(p * 128, (p + 1) * 128)

                qT_sb = qkvp.tile([128, N], BF16, tag="qT")
                kT_sb = qkvp.tile([128, N], BF16, tag="kT")
                v_sb = qkvp.tile([128, N], BF16, tag="v")

                # qT/kT: out[dpair 128, seq 512] = wq_chunk.T @ xT_chunk
                for dst, wname in ((qT_sb, "wq"), (kT_sb, "wk")):
                    for n in range(NT):
                        ps = ps_p_pool.tile([128, 512], F32, tag="proj")
                        for kk in range(8):
                            nc.tensor.matmul(
                                ps[:], w_sb[wname][kk][:, pc],
                                xT_sb[kk][:, n * 512:(n + 1) * 512],
                                start=(kk == 0), stop=(kk == 7))
                        nc.vector.tensor_copy(dst[:, n * 512:(n + 1) * 512], ps[:])
                # v (natural layout, seq on partitions, packed as 16 col-blocks):
                # out[seq 128, dpair 128] = xT_chunk.T @ wv_chunk
                for s in range(16):
                    ps = ps_p_pool.tile([128, 512], F32, tag="proj")
                    for kk in range(8):
                        nc.tensor.matmul(
                            ps[:, 0:128], xT_sb[kk][:, s * 128:(s + 1) * 128],
                            w_sb["wv"][kk][:, pc],
                            start=(kk == 0), stop=(kk == 7))
                    nc.vector.tensor_copy(v_sb[:, s * 128:(s + 1) * 128], ps[:, 0:128])

                # ---- attention (flash, q-tile outer, k-chunk inner) ----
                for t in range(NT):
                    nkc = 4 * t + 4  # causal: k chunks 0 .. 4t+3
                    ps_c = ps_c_pool.tile([128, 512], F32, tag="ctx")
                    ps_m = ps_m_pool.tile([33, 512], F32, tag="mb")
                    for j in range(nkc):
                        last = (j == nkc - 1)
                        ps_sc = ps_s_pool.tile([128, 1024], F32, tag="scores")
                        # scoresT[k,q] pair-packed on row groups
                        nc.tensor.matmul(
                            ps_sc[:, 0:512], kT_sb[0:64, j * 128:(j + 1) * 128],
                            qT_sb[0:64, t * 512:(t + 1) * 512],
                            start=True, stop=True, tile_position=(0, 0))
                        nc.tensor.matmul(
                            ps_sc[:, 512:1024], kT_sb[64:128, j * 128:(j + 1) * 128],
                            qT_sb[64:128, t * 512:(t + 1) * 512],
                            start=True, stop=True, tile_position=(64, 0))
                        e = ep.tile([128, 1024], BF16, tag="e")
                        nc.scalar.activation(e[:], ps_sc[:], EXP, scale=SCALE)
                        if j // 4 == t:  # diagonal-boundary chunk: causal mask
                            o = j % 4
                            m = mask_sb[:, o * 512:(o + 1) * 512]
                            nc.vector.tensor_tensor(e[:, 0:512], e[:, 0:512], m, op=MULT)
                            nc.vector.tensor_tensor(e[:, 512:1024], e[:, 512:1024], m, op=MULT)
                        # ctx accumulation, col-packed pair
                        nc.tensor.matmul(
                            ps_c[0:64, :], v_sb[:, j * 128:j * 128 + 64], e[:, 0:512],
                            start=(j == 0), stop=last, tile_position=(0, 0))
                        nc.tensor.matmul(
                            ps_c[64:128, :], v_sb[:, j * 128 + 64:(j + 1) * 128],
                            e[:, 512:1024],
                            start=(j == 0), stop=last, tile_position=(0, 64))
                        # softmax denominators via ones-matmul
                        nc.tensor.matmul(
                            ps_m[0:1, :], ones_bf[:, 0:1], e[:, 0:512],
                            start=(j == 0), stop=last, tile_position=(0, 0))
                        nc.tensor.matmul(
                            ps_m[32:33, :], ones_bf[:, 0:1], e[:, 512:1024],
                            start=(j == 0), stop=last, tile_position=(0, 32))

                    # evacuate ctx quickly (frees the PSUM bank), normalize later
                    ctxu = evp.tile([128, 512], BF16, tag="ctxu")
                    nc.vector.tensor_copy(ctxu[:], ps_c[:])

                    inv = sp.tile([33, 512], F32, tag="inv")
                    nc.vector.reciprocal(inv[0:1, :], ps_m[0:1, :])
                    nc.vector.reciprocal(inv[32:33, :], ps_m[32:33, :])
                    # broadcast inv over partitions with K=1 matmuls
                    ps_b = ps_m_pool.tile([128, 512], F32, tag="mb")
                    nc.tensor.matmul(ps_b[0:64, :], ones_f[0:1, :], inv[0:1, :],
                                     start=True, stop=True, tile_position=(0, 0))
                    nc.tensor.matmul(ps_b[64:128, :], ones_f[32:33, :], inv[32:33, :],
                                     start=True, stop=True, tile_position=(32, 64))
                    invb = sp.tile([128, 512], F32, tag="invb")
                    nc.vector.tensor_copy(invb[:], ps_b[:])
                    nc.vector.tensor_tensor(
                        ctxT_sb[p][:, t * 512:(t + 1) * 512], ctxu[:], invb[:], op=MULT)

            # ---- output projection (transposed): outT[o, q] = Wo.T-ish ----
            for m in range(8):
                for n in range(NT):
                    ps = ps_p_pool.tile([128, 512], F32, tag="proj")
                    for p in range(NPAIRS):
                        nc.tensor.matmul(
                            ps[:], wo_sb[p][:, m * 128:(m + 1) * 128],
                            ctxT_sb[p][:, n * 512:(n + 1) * 512],
                            start=(p == 0), stop=(p == 3))
                    osb = evp.tile([128, 512], F32, tag="osb")
                    nc.vector.tensor_copy(osb[:], ps[:])
                    nc.sync.dma_start(
                        outT[m * 128:(m + 1) * 128, n * 512:(n + 1) * 512], osb[:])

    nc.compile()
    return nc


_NC = None


def _get_program():
    global _NC
    if _NC is None:
        _NC = _build_program()
    return _NC


def _make_masks():
    r = np.arange(128)[:, None]
    c = np.arange(512)[None, :]
    m = np.zeros((128, 2048), dtype=ml_dtypes.bfloat16)
    for o in range(4):
        m[:, o * 512:(o + 1) * 512] = (c >= o * 128 + r).astype(ml_dtypes.bfloat16)
    return m


def kernel(inputs, Wq, Wk, Wv, Wo, bo):
    inputs = np.asarray(inputs, dtype=np.float32)
    Wq = np.asarray(Wq, dtype=np.float32)
    Wk = np.asarray(Wk, dtype=np.float32)
    Wv = np.asarray(Wv, dtype=np.float32)
    Wo = np.asarray(Wo, dtype=np.float32)
    bo = np.asarray(bo, dtype=np.float32)

    nc = _get_program()
    bf = ml_dtypes.bfloat16
    masks_np = _make_masks()

    in_maps = []
    for core in range(8):
        b, g = core // 2, core % 2
        gs = slice(g * D_LOC, (g + 1) * D_LOC)
        in_maps.append({
            "xT": np.ascontiguousarray(inputs[b].T).astype(bf),
            "wq": Wq[:, gs].astype(bf),
            "wk": Wk[:, gs].astype(bf),
            "wv": Wv[:, gs].astype(bf),
            "wo": np.ascontiguousarray(Wo[gs, :]).astype(bf),
            "masks": masks_np,
        })

    res = run_bass_kernel_spmd(nc, in_maps, core_ids=list(range(8)))
    out = np.empty((4, N, D_IN), dtype=np.float32)
    for b in range(4):
        acc = res.results[2 * b]["outT"] + res.results[2 * b + 1]["outT"]
        out[b] = acc.T + bo[None, :]
    return out


# revision 7
# speedup vs baseline: 23.4061x; 23.4061x over previous
"""Multi-head causal self-attention (b=4, n=2048, d=1024, 16 heads) on 8 TRN2 cores.

Sharding: core i handles batch b = i//2 and head-group g = i%2 (8 heads each).
Per core: QKV projections for its head-group, flash-style causal attention in
scoresT [k, q] layout (bf16 matmuls, fp32 PSUM accum, head pairs packed onto
the PE array via tile_position), and a partial transposed output projection.
Host: out[b] = (outT[2b] + outT[2b+1]).T + bo.
"""
import sys

for _p in ("/opt/trn_rl_repo", "/root/.axon_site", "/root/.axon_site/_ro/trn_rl_repo",
           "/root/.axon_site/_ro/pypackages"):
    if _p not in sys.path:
        sys.path.append(_p)

import numpy as np
import ml_dtypes

import concourse.bass as bass
import concourse.bacc as bacc
import concourse.tile as tile
from concourse import mybir
from concourse.bass_utils import run_bass_kernel_spmd

BF16 = mybir.dt.bfloat16
F32 = mybir.dt.float32

N = 2048          # sequence length
D_IN = 1024       # model dim
D_LOC = 512       # per-core head-group width (8 heads * 64)
NPAIRS = 4        # head pairs per core
NT = 4            # q tiles of 512
SCALE = 1.0 / 8.0  # 1/sqrt(head_dim)

EXP = mybir.ActivationFunctionType.Exp
LOG = mybir.ActivationFunctionType.Ln
MULT = mybir.AluOpType.mult


def _build_program():
    nc = bacc.Bacc("TRN2", target_bir_lowering=False, debug=False, num_devices=8)

    xT = nc.dram_tensor("xT", [D_IN, N], BF16, kind="ExternalInput").ap()
    wq = nc.dram_tensor("wq", [D_IN, D_LOC], BF16, kind="ExternalInput").ap()
    wk = nc.dram_tensor("wk", [D_IN, D_LOC], BF16, kind="ExternalInput").ap()
    wv = nc.dram_tensor("wv", [D_IN, D_LOC], BF16, kind="ExternalInput").ap()
    wo = nc.dram_tensor("wo", [D_LOC, D_IN], BF16, kind="ExternalInput").ap()
    masks = nc.dram_tensor("masks", [128, 128], BF16, kind="ExternalInput").ap()
    outT = nc.dram_tensor("outT", [D_IN, N], F32, kind="ExternalOutput").ap()

    with tile.TileContext(nc) as tc:
        with tc.tile_pool(name="persist", bufs=1) as pp, \
             tc.tile_pool(name="qkv", bufs=2) as qkvp, \
             tc.tile_pool(name="exp", bufs=3) as ep, \
             tc.tile_pool(name="small", bufs=2) as sp, \
             tc.tile_pool(name="evac", bufs=3) as evp, \
             tc.tile_pool(name="ps_s", bufs=2, space="PSUM") as ps_s_pool, \
             tc.tile_pool(name="ps_c", bufs=1, space="PSUM") as ps_c_pool, \
             tc.tile_pool(name="ps_m", bufs=1, space="PSUM") as ps_m_pool, \
             tc.tile_pool(name="ps_p", bufs=2, space="PSUM") as ps_p_pool:

            # ---- persistent SBUF loads ----
            xT_sb = []
            for i in range(8):
                t = pp.tile([128, N], BF16, tag=f"xT{i}", name=f"xT{i}")
                nc.sync.dma_start(t[:], xT[i * 128:(i + 1) * 128, :])
                xT_sb.append(t)
            w_sb = {}
            for name, src in (("wq", wq), ("wk", wk), ("wv", wv)):
                tiles = []
                for i in range(8):
                    t = pp.tile([128, D_LOC], BF16, tag=f"{name}{i}", name=f"{name}{i}")
                    nc.sync.dma_start(t[:], src[i * 128:(i + 1) * 128, :])
                    tiles.append(t)
                w_sb[name] = tiles
            wo_sb = []
            for i in range(4):
                t = pp.tile([128, D_IN], BF16, tag=f"wo{i}", name=f"wo{i}")
                nc.sync.dma_start(t[:], wo[i * 128:(i + 1) * 128, :])
                wo_sb.append(t)
            mask_sb = pp.tile([128, 128], BF16, tag="masks")
            nc.sync.dma_start(mask_sb[:], masks[:])

            ones_col = pp.tile([128, 1], BF16, tag="ones_col")
            nc.vector.memset(ones_col[:], 1.0)
            ones_row = pp.tile([33, 64], BF16, tag="ones_row")
            nc.vector.memset(ones_row[:], 1.0)

            ctxT_sb = [pp.tile([128, N], BF16, tag=f"ctxT{p}", name=f"ctxT{p}")
                       for p in range(NPAIRS)]

            # ---- V projection for all pairs at once (full-width N=512 matmuls)
            # v_all layout: [128, 16*512]; seq chunk c, local d column:
            # v_all[r, c*512 + d] = v[c*128 + r, d]
            v_all = pp.tile([128, 16 * 512], BF16, tag="v_all")
            for c in range(16):
                ps = ps_p_pool.tile([128, 512], F32, tag="proj")
                for kk in range(8):
                    nc.tensor.matmul(
                        ps[:], xT_sb[kk][:, c * 128:(c + 1) * 128],
                        w_sb["wv"][kk][:],
                        start=(kk == 0), stop=(kk == 7))
                nc.vector.tensor_copy(v_all[:, c * 512:(c + 1) * 512], ps[:])

            # ---- per head-pair: q/k projections then attention ----
            for p in range(NPAIRS):
                pc = slice(p * 128, (p + 1) * 128)

                qT_sb = qkvp.tile([128, N], BF16, tag="qT")
                kT_sb = qkvp.tile([128, N], BF16, tag="kT")
                for dst, wname in ((qT_sb, "wq"), (kT_sb, "wk")):
                    for n in range(NT):
                        ps = ps_p_pool.tile([128, 512], F32, tag="proj")
                        for kk in range(8):
                            nc.tensor.matmul(
                                ps[:], w_sb[wname][kk][:, pc],
                                xT_sb[kk][:, n * 512:(n + 1) * 512],
                                start=(kk == 0), stop=(kk == 7))
                        nc.vector.tensor_copy(dst[:, n * 512:(n + 1) * 512], ps[:])

                def vsl(j, h):
                    # lhsT [128 seq, 64] for k-chunk j, head-half h of pair p
                    return v_all[:, j * 512 + p * 128 + h * 64:
                                 j * 512 + p * 128 + (h + 1) * 64]

                # ---- attention (flash, q-tile outer, k-chunk inner) ----
                for t in range(NT):
                    nkc = 4 * t + 4  # causal: k chunks 0 .. 4t+3
                    ps_c = ps_c_pool.tile([128, 512], F32, tag="ctx")
                    ps_m = ps_m_pool.tile([33, 512], F32, tag="mb")
                    for j in range(nkc):
                        last = (j == nkc - 1)
                        bnd = (j // 4 == t)
                        o = j % 4
                        # boundary chunks only attend to q columns >= o*128
                        q0 = o * 128 if bnd else 0
                        qw = 512 - q0
                        qs = slice(t * 512 + q0, (t + 1) * 512)
                        ps_sc = ps_s_pool.tile([128, 1024], F32, tag="scores")
                        # scoresT[k,q] pair-packed on PE row groups
                        nc.tensor.matmul(
                            ps_sc[:, q0:512], kT_sb[0:64, j * 128:(j + 1) * 128],
                            qT_sb[0:64, qs],
                            start=True, stop=True, tile_position=(0, 0))
                        nc.tensor.matmul(
                            ps_sc[:, 512 + q0:1024], kT_sb[64:128, j * 128:(j + 1) * 128],
                            qT_sb[64:128, qs],
                            start=True, stop=True, tile_position=(64, 0))
                        e = ep.tile([128, 1024], BF16, tag="e")
                        if bnd:
                            src = ps_sc[:].rearrange("p (c w) -> p c w", c=2)[:, :, q0:512]
                            dst = e[:].rearrange("p (c w) -> p c w", c=2)[:, :, q0:512]
                            nc.scalar.activation(dst, src, EXP, scale=SCALE)
                            # triangular mask on the 128-wide diagonal block
                            mdst = e[:].rearrange("p (c w) -> p c w", c=2)[:, :, q0:q0 + 128]
                            msrc = mask_sb[:]
                            msrc2 = bass.AP(msrc.tensor, msrc.offset,
                                            [list(msrc.ap[0]), [0, 2], [1, 128]])
                            nc.vector.tensor_tensor(mdst, mdst, msrc2, op=MULT)
                        else:
                            nc.scalar.activation(e[:], ps_sc[:], EXP, scale=SCALE)
                        # ctx accumulation, col-packed pair
                        nc.tensor.matmul(
                            ps_c[0:64, q0:512], vsl(j, 0), e[:, q0:512],
                            start=(j == 0), stop=last, tile_position=(0, 0))
                        nc.tensor.matmul(
                            ps_c[64:128, q0:512], vsl(j, 1), e[:, 512 + q0:1024],
                            start=(j == 0), stop=last, tile_position=(0, 64))
                        # softmax denominators via ones-matmul
                        nc.tensor.matmul(
                            ps_m[0:1, q0:512], ones_col[:, 0:1], e[:, q0:512],
                            start=(j == 0), stop=last, tile_position=(0, 0))
                        nc.tensor.matmul(
                            ps_m[32:33, q0:512], ones_col[:, 0:1], e[:, 512 + q0:1024],
                            start=(j == 0), stop=last, tile_position=(0, 32))

                    # evacuate ctx quickly (frees the PSUM bank), normalize after
                    ctxu = evp.tile([128, 512], BF16, tag="ctxu")
                    nc.vector.tensor_copy(ctxu[:], ps_c[:])

                    # 1/s = exp(-log(s)) on ACT: the 1-lane DVE reciprocal is
                    # ~3.3us per call; Log+Exp share one ACT table set.
                    lns = sp.tile([33, 512], F32, tag="lns")
                    nc.scalar.activation(lns[0:1, :], ps_m[0:1, :], LOG)
                    nc.scalar.activation(lns[32:33, :], ps_m[32:33, :], LOG)
                    invh = sp.tile([33, 512], BF16, tag="invh")
                    nc.scalar.activation(invh[0:1, :], lns[0:1, :], EXP, scale=-1.0)
                    nc.scalar.activation(invh[32:33, :], lns[32:33, :], EXP, scale=-1.0)
                    # broadcast inv over partitions with K=1 bf16 matmuls
                    ps_b = ps_m_pool.tile([128, 512], F32, tag="mb")
                    nc.tensor.matmul(ps_b[0:64, :], ones_row[0:1, :], invh[0:1, :],
                                     start=True, stop=True, tile_position=(0, 0))
                    nc.tensor.matmul(ps_b[64:128, :], ones_row[32:33, :], invh[32:33, :],
                                     start=True, stop=True, tile_position=(32, 64))
                    nc.vector.tensor_tensor(
                        ctxT_sb[p][:, t * 512:(t + 1) * 512], ctxu[:], ps_b[:], op=MULT)

            # ---- output projection (transposed): outT[o_col, q] ----
            for m in range(8):
                for n in range(NT):
                    ps = ps_p_pool.tile([128, 512], F32, tag="proj")
                    for p in range(NPAIRS):
                        nc.tensor.matmul(
                            ps[:], wo_sb[p][:, m * 128:(m + 1) * 128],
                            ctxT_sb[p][:, n * 512:(n + 1) * 512],
                            start=(p == 0), stop=(p == 3))
                    osb = evp.tile([128, 512], F32, tag="osb")
                    nc.vector.tensor_copy(osb[:], ps[:])
                    nc.sync.dma_start(
                        outT[m * 128:(m + 1) * 128, n * 512:(n + 1) * 512], osb[:])

    nc.compile()
    return nc


_NC = None


def _get_program():
    global _NC
    if _NC is None:
        _NC = _build_program()
    return _NC


def _make_masks():
    r = np.arange(128)[:, None]
    c = np.arange(128)[None, :]
    return (c >= r).astype(ml_dtypes.bfloat16)


def kernel(inputs, Wq, Wk, Wv, Wo, bo):
    inputs = np.asarray(inputs, dtype=np.float32)
    Wq = np.asarray(Wq, dtype=np.float32)
    Wk = np.asarray(Wk, dtype=np.float32)
    Wv = np.asarray(Wv, dtype=np.float32)
    Wo = np.asarray(Wo, dtype=np.float32)
    bo = np.asarray(bo, dtype=np.float32)

    nc = _get_program()
    bf = ml_dtypes.bfloat16
    masks_np = _make_masks()

    in_maps = []
    for core in range(8):
        b, g = core // 2, core % 2
        gs = slice(g * D_LOC, (g + 1) * D_LOC)
        in_maps.append({
            "xT": np.ascontiguousarray(inputs[b].T).astype(bf),
            "wq": Wq[:, gs].astype(bf),
            "wk": Wk[:, gs].astype(bf),
            "wv": Wv[:, gs].astype(bf),
            "wo": np.ascontiguousarray(Wo[gs, :]).astype(bf),
            "masks": masks_np,
        })

    res = run_bass_kernel_spmd(nc, in_maps, core_ids=list(range(8)))
    out = np.empty((4, N, D_IN), dtype=np.float32)
    for b in range(4):
        acc = res.results[2 * b]["outT"] + res.results[2 * b + 1]["outT"]
        out[b] = acc.T + bo[None, :]
    return out


# revision 8
# speedup vs baseline: 27.5036x; 1.1751x over previous
"""Multi-head causal self-attention (b=4, n=2048, d=1024, 16 heads) on 8 TRN2 cores.

Sharding: core i handles batch b = i//2 and head-group g = i%2 (8 heads each).
Per core: QKV projections for its head-group, flash-style causal attention in
scoresT [k, q] layout (bf16 matmuls, fp32 PSUM accum, head pairs packed onto
the PE array via tile_position), and a partial transposed output projection.
Host: out[b] = (outT[2b] + outT[2b+1]).T + bo.
"""
import sys

for _p in ("/opt/trn_rl_repo", "/root/.axon_site", "/root/.axon_site/_ro/trn_rl_repo",
           "/root/.axon_site/_ro/pypackages"):
    if _p not in sys.path:
        sys.path.append(_p)

import numpy as np
import ml_dtypes

import concourse.bass as bass
import concourse.bacc as bacc
import concourse.tile as tile
from concourse import mybir
from concourse.bass_utils import run_bass_kernel_spmd

BF16 = mybir.dt.bfloat16
F32 = mybir.dt.float32

N = 2048          # sequence length
D_IN = 1024       # model dim
D_LOC = 512       # per-core head-group width (8 heads * 64)
NPAIRS = 4        # head pairs per core
NT = 4            # q tiles of 512
SCALE = 1.0 / 8.0  # 1/sqrt(head_dim)

EXP = mybir.ActivationFunctionType.Exp
LOG = mybir.ActivationFunctionType.Ln
MULT = mybir.AluOpType.mult


def _build_program():
    nc = bacc.Bacc("TRN2", target_bir_lowering=False, debug=False, num_devices=8)

    xT = nc.dram_tensor("xT", [D_IN, N], BF16, kind="ExternalInput").ap()
    wq = nc.dram_tensor("wq", [D_IN, D_LOC], BF16, kind="ExternalInput").ap()
    wk = nc.dram_tensor("wk", [D_IN, D_LOC], BF16, kind="ExternalInput").ap()
    wv = nc.dram_tensor("wv", [D_IN, D_LOC], BF16, kind="ExternalInput").ap()
    wo = nc.dram_tensor("wo", [D_LOC, D_IN], BF16, kind="ExternalInput").ap()
    masks = nc.dram_tensor("masks", [128, 128], BF16, kind="ExternalInput").ap()
    outT = nc.dram_tensor("outT", [D_IN, N], F32, kind="ExternalOutput").ap()

    with tile.TileContext(nc) as tc:
        with tc.tile_pool(name="persist", bufs=1) as pp, \
             tc.tile_pool(name="qkv", bufs=2) as qkvp, \
             tc.tile_pool(name="exp", bufs=3) as ep, \
             tc.tile_pool(name="small", bufs=2) as sp, \
             tc.tile_pool(name="evac", bufs=3) as evp, \
             tc.tile_pool(name="ps_s", bufs=2, space="PSUM") as ps_s_pool, \
             tc.tile_pool(name="ps_c", bufs=1, space="PSUM") as ps_c_pool, \
             tc.tile_pool(name="ps_m", bufs=1, space="PSUM") as ps_m_pool, \
             tc.tile_pool(name="ps_p", bufs=2, space="PSUM") as ps_p_pool:

            # ---- persistent SBUF loads ----
            xT_sb = []
            for i in range(8):
                t = pp.tile([128, N], BF16, tag=f"xT{i}", name=f"xT{i}")
                nc.sync.dma_start(t[:], xT[i * 128:(i + 1) * 128, :])
                xT_sb.append(t)
            w_sb = {}
            for name, src in (("wq", wq), ("wk", wk), ("wv", wv)):
                tiles = []
                for i in range(8):
                    t = pp.tile([128, D_LOC], BF16, tag=f"{name}{i}", name=f"{name}{i}")
                    nc.sync.dma_start(t[:], src[i * 128:(i + 1) * 128, :])
                    tiles.append(t)
                w_sb[name] = tiles
            wo_sb = []
            for i in range(4):
                t = pp.tile([128, D_IN], BF16, tag=f"wo{i}", name=f"wo{i}")
                nc.sync.dma_start(t[:], wo[i * 128:(i + 1) * 128, :])
                wo_sb.append(t)
            mask_sb = pp.tile([128, 128], BF16, tag="masks")
            nc.sync.dma_start(mask_sb[:], masks[:])

            ones_col = pp.tile([128, 1], BF16, tag="ones_col")
            nc.vector.memset(ones_col[:], 1.0)
            ones_row = pp.tile([33, 64], BF16, tag="ones_row")
            nc.vector.memset(ones_row[:], 1.0)

            ctxT_sb = [pp.tile([128, N], BF16, tag=f"ctxT{p}", name=f"ctxT{p}")
                       for p in range(NPAIRS)]
            sums_sb = [pp.tile([33, N], F32, tag=f"sums{p}", name=f"sums{p}")
                       for p in range(NPAIRS)]

            # ---- V projection for all pairs at once (full-width N=512 matmuls)
            # v_all layout: [128, 16*512]; seq chunk c, local d column:
            # v_all[r, c*512 + d] = v[c*128 + r, d]
            v_all = pp.tile([128, 16 * 512], BF16, tag="v_all")
            for c in range(16):
                ps = ps_p_pool.tile([128, 512], F32, tag="proj")
                for kk in range(8):
                    nc.tensor.matmul(
                        ps[:], xT_sb[kk][:, c * 128:(c + 1) * 128],
                        w_sb["wv"][kk][:],
                        start=(kk == 0), stop=(kk == 7))
                nc.vector.tensor_copy(v_all[:, c * 512:(c + 1) * 512], ps[:])

            # ---- per head-pair: q/k projections then attention ----
            for p in range(NPAIRS):
                pc = slice(p * 128, (p + 1) * 128)

                qT_sb = qkvp.tile([128, N], BF16, tag="qT")
                kT_sb = qkvp.tile([128, N], BF16, tag="kT")
                for dst, wname in ((qT_sb, "wq"), (kT_sb, "wk")):
                    for n in range(NT):
                        ps = ps_p_pool.tile([128, 512], F32, tag="proj")
                        for kk in range(8):
                            nc.tensor.matmul(
                                ps[:], w_sb[wname][kk][:, pc],
                                xT_sb[kk][:, n * 512:(n + 1) * 512],
                                start=(kk == 0), stop=(kk == 7))
                        nc.vector.tensor_copy(dst[:, n * 512:(n + 1) * 512], ps[:])

                def vsl(j, h):
                    # lhsT [128 seq, 64] for k-chunk j, head-half h of pair p
                    return v_all[:, j * 512 + p * 128 + h * 64:
                                 j * 512 + p * 128 + (h + 1) * 64]

                # ---- attention (flash, q-tile outer, k-chunk inner) ----
                for t in range(NT):
                    nkc = 4 * t + 4  # causal: k chunks 0 .. 4t+3
                    ps_c = ps_c_pool.tile([128, 512], F32, tag="ctx")
                    ps_m = ps_m_pool.tile([33, 512], F32, tag="mb")
                    for j in range(nkc):
                        last = (j == nkc - 1)
                        bnd = (j // 4 == t)
                        o = j % 4
                        # boundary chunks only attend to q columns >= o*128
                        q0 = o * 128 if bnd else 0
                        qw = 512 - q0
                        qs = slice(t * 512 + q0, (t + 1) * 512)
                        ps_sc = ps_s_pool.tile([128, 1024], F32, tag="scores")
                        # scoresT[k,q] pair-packed on PE row groups
                        nc.tensor.matmul(
                            ps_sc[:, q0:512], kT_sb[0:64, j * 128:(j + 1) * 128],
                            qT_sb[0:64, qs],
                            start=True, stop=True, tile_position=(0, 0))
                        nc.tensor.matmul(
                            ps_sc[:, 512 + q0:1024], kT_sb[64:128, j * 128:(j + 1) * 128],
                            qT_sb[64:128, qs],
                            start=True, stop=True, tile_position=(64, 0))
                        e = ep.tile([128, 1024], BF16, tag="e")
                        if bnd:
                            src = ps_sc[:].rearrange("p (c w) -> p c w", c=2)[:, :, q0:512]
                            dst = e[:].rearrange("p (c w) -> p c w", c=2)[:, :, q0:512]
                            nc.scalar.activation(dst, src, EXP, scale=SCALE)
                            # triangular mask on the 128-wide diagonal block
                            mdst = e[:].rearrange("p (c w) -> p c w", c=2)[:, :, q0:q0 + 128]
                            msrc = mask_sb[:]
                            msrc2 = bass.AP(msrc.tensor, msrc.offset,
                                            [list(msrc.ap[0]), [0, 2], [1, 128]])
                            nc.vector.tensor_tensor(mdst, mdst, msrc2, op=MULT)
                        else:
                            nc.scalar.activation(e[:], ps_sc[:], EXP, scale=SCALE)
                        # ctx accumulation, col-packed pair
                        nc.tensor.matmul(
                            ps_c[0:64, q0:512], vsl(j, 0), e[:, q0:512],
                            start=(j == 0), stop=last, tile_position=(0, 0))
                        nc.tensor.matmul(
                            ps_c[64:128, q0:512], vsl(j, 1), e[:, 512 + q0:1024],
                            start=(j == 0), stop=last, tile_position=(0, 64))
                        # softmax denominators via ones-matmul
                        nc.tensor.matmul(
                            ps_m[0:1, q0:512], ones_col[:, 0:1], e[:, q0:512],
                            start=(j == 0), stop=last, tile_position=(0, 0))
                        nc.tensor.matmul(
                            ps_m[32:33, q0:512], ones_col[:, 0:1], e[:, 512 + q0:1024],
                            start=(j == 0), stop=last, tile_position=(0, 32))

                    # evacuate raw ctx and sums; normalization is deferred to a
                    # single phase so softmax EXPs don't thrash ACT table sets
                    nc.vector.tensor_copy(ctxT_sb[p][:, t * 512:(t + 1) * 512], ps_c[:])
                    nc.vector.tensor_copy(sums_sb[p][:, t * 512:(t + 1) * 512], ps_m[:])

            # ---- deferred softmax normalization (batched Ln/Exp on ACT) ----
            for t in range(NT):
                for p in range(NPAIRS):
                    ts_ = slice(t * 512, (t + 1) * 512)
                    lns = sp.tile([33, 512], F32, tag="lns")
                    nc.scalar.activation(lns[0:1, :], sums_sb[p][0:1, ts_], LOG)
                    nc.scalar.activation(lns[32:33, :], sums_sb[p][32:33, ts_], LOG)
                    invh = sp.tile([33, 512], BF16, tag="invh")
                    nc.scalar.activation(invh[0:1, :], lns[0:1, :], EXP, scale=-1.0)
                    nc.scalar.activation(invh[32:33, :], lns[32:33, :], EXP, scale=-1.0)
                    ps_b = ps_m_pool.tile([128, 512], F32, tag="mb")
                    nc.tensor.matmul(ps_b[0:64, :], ones_row[0:1, :], invh[0:1, :],
                                     start=True, stop=True, tile_position=(0, 0))
                    nc.tensor.matmul(ps_b[64:128, :], ones_row[32:33, :], invh[32:33, :],
                                     start=True, stop=True, tile_position=(32, 64))
                    nc.vector.tensor_tensor(
                        ctxT_sb[p][:, ts_], ctxT_sb[p][:, ts_], ps_b[:], op=MULT)

            # ---- output projection (transposed): outT[o_col, q] ----
            for m in range(8):
                for n in range(NT):
                    ps = ps_p_pool.tile([128, 512], F32, tag="proj")
                    for p in range(NPAIRS):
                        nc.tensor.matmul(
                            ps[:], wo_sb[p][:, m * 128:(m + 1) * 128],
                            ctxT_sb[p][:, n * 512:(n + 1) * 512],
                            start=(p == 0), stop=(p == 3))
                    osb = evp.tile([128, 512], F32, tag="osb")
                    nc.vector.tensor_copy(osb[:], ps[:])
                    nc.sync.dma_start(
                        outT[m * 128:(m + 1) * 128, n * 512:(n + 1) * 512], osb[:])

    nc.compile()
    return nc


_NC = None


def _get_program():
    global _NC
    if _NC is None:
        _NC = _build_program()
    return _NC


def _make_masks():
    r = np.arange(128)[:, None]
    c = np.arange(128)[None, :]
    return (c >= r).astype(ml_dtypes.bfloat16)


def kernel(inputs, Wq, Wk, Wv, Wo, bo):
    inputs = np.asarray(inputs, dtype=np.float32)
    Wq = np.asarray(Wq, dtype=np.float32)
    Wk = np.asarray(Wk, dtype=np.float32)
    Wv = np.asarray(Wv, dtype=np.float32)
    Wo = np.asarray(Wo, dtype=np.float32)
    bo = np.asarray(bo, dtype=np.float32)

    nc = _get_program()
    bf = ml_dtypes.bfloat16
    masks_np = _make_masks()

    in_maps = []
    for core in range(8):
        b, g = core // 2, core % 2
        gs = slice(g * D_LOC, (g + 1) * D_LOC)
        in_maps.append({
            "xT": np.ascontiguousarray(inputs[b].T).astype(bf),
            "wq": Wq[:, gs].astype(bf),
            "wk": Wk[:, gs].astype(bf),
            "wv": Wv[:, gs].astype(bf),
            "wo": np.ascontiguousarray(Wo[gs, :]).astype(bf),
            "masks": masks_np,
        })

    res = run_bass_kernel_spmd(nc, in_maps, core_ids=list(range(8)))
    out = np.empty((4, N, D_IN), dtype=np.float32)
    for b in range(4):
        acc = res.results[2 * b]["outT"] + res.results[2 * b + 1]["outT"]
        out[b] = acc.T + bo[None, :]
    return out


# revision 9
# speedup vs baseline: 29.0671x; 1.0568x over previous
"""Multi-head causal self-attention (b=4, n=2048, d=1024, 16 heads) on 8 TRN2 cores.

Sharding: core i handles batch b = i//2 and head-group g = i%2 (8 heads each).
Per core: QKV projections for its head-group, flash-style causal attention in
scoresT [k, q] layout (bf16 matmuls, fp32 PSUM accum, head pairs packed onto
the PE array via tile_position), and a partial transposed output projection.
Host: out[b] = (outT[2b] + outT[2b+1]).T + bo.
"""
import sys

for _p in ("/opt/trn_rl_repo", "/root/.axon_site", "/root/.axon_site/_ro/trn_rl_repo",
           "/root/.axon_site/_ro/pypackages"):
    if _p not in sys.path:
        sys.path.append(_p)

import numpy as np
import ml_dtypes

import concourse.bass as bass
import concourse.bacc as bacc
import concourse.tile as tile
from concourse import mybir
from concourse.bass_utils import run_bass_kernel_spmd

BF16 = mybir.dt.bfloat16
F32 = mybir.dt.float32

N = 2048          # sequence length
D_IN = 1024       # model dim
D_LOC = 512       # per-core head-group width (8 heads * 64)
NPAIRS = 4        # head pairs per core
NT = 4            # q tiles of 512
SCALE = 1.0 / 8.0  # 1/sqrt(head_dim)

EXP = mybir.ActivationFunctionType.Exp
LOG = mybir.ActivationFunctionType.Ln
MULT = mybir.AluOpType.mult


def _build_program():
    nc = bacc.Bacc("TRN2", target_bir_lowering=False, debug=False, num_devices=8)

    xT = nc.dram_tensor("xT", [D_IN, N], BF16, kind="ExternalInput").ap()
    wq = nc.dram_tensor("wq", [D_IN, D_LOC], BF16, kind="ExternalInput").ap()
    wk = nc.dram_tensor("wk", [D_IN, D_LOC], BF16, kind="ExternalInput").ap()
    wv = nc.dram_tensor("wv", [D_IN, D_LOC], BF16, kind="ExternalInput").ap()
    wo = nc.dram_tensor("wo", [D_LOC, D_IN], BF16, kind="ExternalInput").ap()
    masks = nc.dram_tensor("masks", [128, 128], BF16, kind="ExternalInput").ap()
    outT = nc.dram_tensor("outT", [D_IN, N], F32, kind="ExternalOutput").ap()

    with tile.TileContext(nc) as tc:
        with tc.tile_pool(name="persist", bufs=1) as pp, \
             tc.tile_pool(name="qkv", bufs=2) as qkvp, \
             tc.tile_pool(name="exp", bufs=3) as ep, \
             tc.tile_pool(name="small", bufs=2) as sp, \
             tc.tile_pool(name="evac", bufs=3) as evp, \
             tc.tile_pool(name="ps_s", bufs=2, space="PSUM") as ps_s_pool, \
             tc.tile_pool(name="ps_c", bufs=1, space="PSUM") as ps_c_pool, \
             tc.tile_pool(name="ps_m", bufs=1, space="PSUM") as ps_m_pool, \
             tc.tile_pool(name="ps_p", bufs=2, space="PSUM") as ps_p_pool:

            # ---- persistent SBUF loads ----
            xT_sb = []
            for i in range(8):
                t = pp.tile([128, N], BF16, tag=f"xT{i}", name=f"xT{i}")
                nc.sync.dma_start(t[:], xT[i * 128:(i + 1) * 128, :])
                xT_sb.append(t)
            w_sb = {}
            for name, src in (("wq", wq), ("wk", wk), ("wv", wv)):
                tiles = []
                for i in range(8):
                    t = pp.tile([128, D_LOC], BF16, tag=f"{name}{i}", name=f"{name}{i}")
                    nc.sync.dma_start(t[:], src[i * 128:(i + 1) * 128, :])
                    tiles.append(t)
                w_sb[name] = tiles
            wo_sb = []
            for i in range(4):
                t = pp.tile([128, D_IN], BF16, tag=f"wo{i}", name=f"wo{i}")
                nc.sync.dma_start(t[:], wo[i * 128:(i + 1) * 128, :])
                wo_sb.append(t)
            mask_sb = pp.tile([128, 128], BF16, tag="masks")
            nc.sync.dma_start(mask_sb[:], masks[:])

            ones_col = pp.tile([128, 1], BF16, tag="ones_col")
            nc.vector.memset(ones_col[:], 1.0)
            ones_row = pp.tile([33, 64], BF16, tag="ones_row")
            nc.vector.memset(ones_row[:], 1.0)

            ctxT_sb = [pp.tile([128, N], BF16, tag=f"ctxT{p}", name=f"ctxT{p}")
                       for p in range(NPAIRS)]
            sums_sb = [pp.tile([33, N], F32, tag=f"sums{p}", name=f"sums{p}")
                       for p in range(NPAIRS)]

            # ---- V projection for all pairs at once (full-width N=512 matmuls)
            # v_all layout: [128, 16*512]; seq chunk c, local d column:
            # v_all[r, c*512 + d] = v[c*128 + r, d]
            v_all = pp.tile([128, 16 * 512], BF16, tag="v_all")
            for c in range(16):
                ps = ps_p_pool.tile([128, 512], F32, tag="proj")
                for kk in range(8):
                    nc.tensor.matmul(
                        ps[:], xT_sb[kk][:, c * 128:(c + 1) * 128],
                        w_sb["wv"][kk][:],
                        start=(kk == 0), stop=(kk == 7))
                nc.vector.tensor_copy(v_all[:, c * 512:(c + 1) * 512], ps[:])

            # ---- per head-pair: q/k projections then attention ----
            for p in range(NPAIRS):
                pc = slice(p * 128, (p + 1) * 128)

                qT_sb = qkvp.tile([128, N], BF16, tag="qT")
                kT_sb = qkvp.tile([128, N], BF16, tag="kT")
                for dst, wname in ((qT_sb, "wq"), (kT_sb, "wk")):
                    for n in range(NT):
                        ps = ps_p_pool.tile([128, 512], F32, tag="proj")
                        for kk in range(8):
                            nc.tensor.matmul(
                                ps[:], w_sb[wname][kk][:, pc],
                                xT_sb[kk][:, n * 512:(n + 1) * 512],
                                start=(kk == 0), stop=(kk == 7))
                        nc.vector.tensor_copy(dst[:, n * 512:(n + 1) * 512], ps[:])

                def vsl(j, h):
                    # lhsT [128 seq, 64] for k-chunk j, head-half h of pair p
                    return v_all[:, j * 512 + p * 128 + h * 64:
                                 j * 512 + p * 128 + (h + 1) * 64]

                # ---- attention (flash, q-tile outer, k-chunk inner) ----
                for t in range(NT):
                    nkc = 4 * t + 4  # causal: k chunks 0 .. 4t+3
                    ps_c = ps_c_pool.tile([128, 512], F32, tag="ctx")
                    ps_m = ps_m_pool.tile([33, 512], F32, tag="mb")
                    for j in range(nkc):
                        last = (j == nkc - 1)
                        bnd = (j // 4 == t)
                        o = j % 4
                        # boundary chunks only attend to q columns >= o*128
                        q0 = o * 128 if bnd else 0
                        qw = 512 - q0
                        qs = slice(t * 512 + q0, (t + 1) * 512)
                        ps_sc = ps_s_pool.tile([128, 1024], F32, tag="scores")
                        # scoresT[k,q] pair-packed on PE row groups
                        nc.tensor.matmul(
                            ps_sc[:, q0:512], kT_sb[0:64, j * 128:(j + 1) * 128],
                            qT_sb[0:64, qs],
                            start=True, stop=True, tile_position=(0, 0))
                        nc.tensor.matmul(
                            ps_sc[:, 512 + q0:1024], kT_sb[64:128, j * 128:(j + 1) * 128],
                            qT_sb[64:128, qs],
                            start=True, stop=True, tile_position=(64, 0))
                        e = ep.tile([128, 1024], BF16, tag="e")
                        if bnd:
                            src = ps_sc[:].rearrange("p (c w) -> p c w", c=2)[:, :, q0:512]
                            dst = e[:].rearrange("p (c w) -> p c w", c=2)[:, :, q0:512]
                            nc.scalar.activation(dst, src, EXP, scale=SCALE)
                            # triangular mask on the 128-wide diagonal block
                            mdst = e[:].rearrange("p (c w) -> p c w", c=2)[:, :, q0:q0 + 128]
                            msrc = mask_sb[:]
                            msrc2 = bass.AP(msrc.tensor, msrc.offset,
                                            [list(msrc.ap[0]), [0, 2], [1, 128]])
                            nc.vector.tensor_tensor(mdst, mdst, msrc2, op=MULT)
                        else:
                            nc.scalar.activation(e[:], ps_sc[:], EXP, scale=SCALE)
                        # ctx accumulation, col-packed pair
                        nc.tensor.matmul(
                            ps_c[0:64, q0:512], vsl(j, 0), e[:, q0:512],
                            start=(j == 0), stop=last, tile_position=(0, 0))
                        nc.tensor.matmul(
                            ps_c[64:128, q0:512], vsl(j, 1), e[:, 512 + q0:1024],
                            start=(j == 0), stop=last, tile_position=(0, 64))
                        # softmax denominators via ones-matmul
                        nc.tensor.matmul(
                            ps_m[0:1, q0:512], ones_col[:, 0:1], e[:, q0:512],
                            start=(j == 0), stop=last, tile_position=(0, 0))
                        nc.tensor.matmul(
                            ps_m[32:33, q0:512], ones_col[:, 0:1], e[:, 512 + q0:1024],
                            start=(j == 0), stop=last, tile_position=(0, 32))

                    # evacuate raw ctx and sums; normalization is deferred to a
                    # single phase so softmax EXPs don't thrash ACT table sets
                    nc.vector.tensor_copy(ctxT_sb[p][:, t * 512:(t + 1) * 512], ps_c[:])
                    nc.vector.tensor_copy(sums_sb[p][:, t * 512:(t + 1) * 512], ps_m[:])

            # ---- deferred softmax normalization ----
            # batched Ln then batched Exp(-x): 2 ACT table loads total, wide ops
            invh_sb = []
            for p in range(NPAIRS):
                nc.scalar.activation(sums_sb[p][0:1, :], sums_sb[p][0:1, :], LOG)
                nc.scalar.activation(sums_sb[p][32:33, :], sums_sb[p][32:33, :], LOG)
            for p in range(NPAIRS):
                ih = sp.tile([33, N], BF16, tag=f"invh{p}", name=f"invh{p}", bufs=1)
                nc.scalar.activation(ih[0:1, :], sums_sb[p][0:1, :], EXP, scale=-1.0)
                nc.scalar.activation(ih[32:33, :], sums_sb[p][32:33, :], EXP, scale=-1.0)
                invh_sb.append(ih)
            for t in range(NT):
                for p in range(NPAIRS):
                    ts_ = slice(t * 512, (t + 1) * 512)
                    ps_b = ps_m_pool.tile([128, 512], F32, tag="mb")
                    nc.tensor.matmul(ps_b[0:64, :], ones_row[0:1, :],
                                     invh_sb[p][0:1, ts_],
                                     start=True, stop=True, tile_position=(0, 0))
                    nc.tensor.matmul(ps_b[64:128, :], ones_row[32:33, :],
                                     invh_sb[p][32:33, ts_],
                                     start=True, stop=True, tile_position=(32, 64))
                    nc.vector.tensor_tensor(
                        ctxT_sb[p][:, ts_], ctxT_sb[p][:, ts_], ps_b[:], op=MULT)

            # ---- output projection (transposed): outT[o_col, q] ----
            for m in range(8):
                for n in range(NT):
                    ps = ps_p_pool.tile([128, 512], F32, tag="proj")
                    for p in range(NPAIRS):
                        nc.tensor.matmul(
                            ps[:], wo_sb[p][:, m * 128:(m + 1) * 128],
                            ctxT_sb[p][:, n * 512:(n + 1) * 512],
                            start=(p == 0), stop=(p == 3))
                    osb = evp.tile([128, 512], F32, tag="osb")
                    nc.vector.tensor_copy(osb[:], ps[:])
                    nc.sync.dma_start(
                        outT[m * 128:(m + 1) * 128, n * 512:(n + 1) * 512], osb[:])

    nc.compile()
    return nc


_NC = None


def _get_program():
    global _NC
    if _NC is None:
        _NC = _build_program()
    return _NC


def _make_masks():
    r = np.arange(128)[:, None]
    c = np.arange(128)[None, :]
    return (c >= r).astype(ml_dtypes.bfloat16)


def kernel(inputs, Wq, Wk, Wv, Wo, bo):
    inputs = np.asarray(inputs, dtype=np.float32)
    Wq = np.asarray(Wq, dtype=np.float32)
    Wk = np.asarray(Wk, dtype=np.float32)
    Wv = np.asarray(Wv, dtype=np.float32)
    Wo = np.asarray(Wo, dtype=np.float32)
    bo = np.asarray(bo, dtype=np.float32)

    nc = _get_program()
    bf = ml_dtypes.bfloat16
    masks_np = _make_masks()

    in_maps = []
    for core in range(8):
        b, g = core // 2, core % 2
        gs = slice(g * D_LOC, (g + 1) * D_LOC)
        in_maps.append({
            "xT": np.ascontiguousarray(inputs[b].T).astype(bf),
            "wq": Wq[:, gs].astype(bf),
            "wk": Wk[:, gs].astype(bf),
            "wv": Wv[:, gs].astype(bf),
            "wo": np.ascontiguousarray(Wo[gs, :]).astype(bf),
            "masks": masks_np,
        })

    res = run_bass_kernel_spmd(nc, in_maps, core_ids=list(range(8)))
    out = np.empty((4, N, D_IN), dtype=np.float32)
    for b in range(4):
        acc = res.results[2 * b]["outT"] + res.results[2 * b + 1]["outT"]
        out[b] = acc.T + bo[None, :]
    return out
